# revision 8
# baseline (speedup 1.0000x reference)
"""Trainium2 Bass kernel for nn_Attention_5299989643989.

GQA attention forward (B=2, T=2048, C=1024, 16 q heads / 4 kv heads, D=64)
with value-embedding gating, rotary embedding, qk rms-norm, causal softmax.

Sharding: 8 cores = batch (2) x kv-head-group (4).  Each core computes its
4 q heads / 1 kv head end-to-end plus the Wo row-shard partial output; the
host sums the 4 partials per batch (the Wo all-reduce, done at unshard).

Structure (all matmuls f32r: 16-bit weights would emit one Ldweights per
matmul, ~71ns of PE sequencer each, which measured slower than the f32r
penalty; PSUM accumulation groups must be emitted contiguously -- the PE
corrupts interleaved open groups in one bank, though nesting a whole group
inside another group in a DIFFERENT bank is fine):
  inputs:  host pre-tiles everything so each DMA is 128 contiguous
           descriptors; x and the jammed q|k|v|gate weight block are bf16
           (halves the DMA-bound startup), cos/sin/ve bf16, out staged bf16.
  phase1:  per 128-token chunk one jammed projection matmul accumulated
           over C in PSUM -> bf16 SBUF; per group: rope on packed bf16 DVE
           ops (2x mode), sum-of-squares + bit-trick Newton rsqrt for the
           rms-norm, sigmoid gate via Tanh (keeps the Exp ACT table
           resident), ve-gating on GPSIMD; f32r transposes (q pairs + k
           with row duplication so odd heads run at partition base 64).
  phase2:  per (block, head): score matmuls emitted in PAIRS into a
           two-bank PSUM tile, ONE exp over [128, 1024] for off-diagonal
           pairs (split per tile on the diagonal to skip masked columns),
           triangular mask on GPSIMD, AV accumulation yp[65, 512] with a
           ones column producing softmax denominators for free.  The AV of
           pair p trails the scores of pair p+2 (software stagger), and the
           pipeline runs CONTINUOUSLY across heads and across the merged
           blocks 2+3 (head-interleaved to average the ACT-heavy late
           blocks against PE-heavy Wo/output work).
  norm:    per head pair one [33, 512] reciprocal_approx_fast (rows 0/32;
           base-0 only -- the op miscomputes at shifted partition bases),
           one selector matmul broadcasts both rows to 128 partitions,
           one in-place scale of the feature-major y.
  out:     row-sharded Wo per 128-token chunk, PSUM -> bf16 staging split
           across DVE/ACT, one big output DMA per 512-token block.
  The whole thing is emission-software-pipelined: projection chunks for
  ALL groups run in the prologue while x blocks stream in; transposes,
  Wo/output work and the next groups' DVE chains are interleaved between
  phase-2 score pairs as "fillers" with readiness gates.
"""

import numpy as np
import ml_dtypes

import concourse.bacc as bacc
import concourse.bass as bass
import concourse.tile as tile
from concourse import mybir

f32 = mybir.dt.float32
f32r = mybir.dt.float32r
bf16 = mybir.dt.bfloat16
u32 = mybir.dt.uint32
AF = mybir.ActivationFunctionType
ALU = mybir.AluOpType

B, T, C = 2, 2048, 1024
N_HEAD, N_KV_HEAD, D = 16, 4, 64
HQ = N_HEAD // N_KV_HEAD  # q heads per core = 4
P = 128
NT = T // P       # 16 token chunks
KC = C // P       # 8 contraction chunks
IB = 512          # query block
NBI = T // IB     # 4 query blocks
GRP = IB // P     # 4 token chunks per query block
SC = 1.2 * 1.2 / 8.0  # folded qk scale: rms 1.2 factors * 1/sqrt(64)
H32 = D // 2
NPJ = 386         # q(256) | k(64) | v(64) | gate(1) | pad(1)


def build_program():
    nc = bacc.Bacc("TRN2", target_bir_lowering=False, debug=False, num_devices=8)

    xT = nc.dram_tensor("xT", [P, NBI, GRP, KC, P], bf16, kind="ExternalInput")
    wr = nc.dram_tensor("wr", [P, KC, NPJ], bf16, kind="ExternalInput")
    cosd = nc.dram_tensor("cosd", [P, NT, H32], bf16, kind="ExternalInput")
    sind = nc.dram_tensor("sind", [P, NT, H32], bf16, kind="ExternalInput")
    ve3 = nc.dram_tensor("ve3", [P, NT, D], bf16, kind="ExternalInput")
    woT = nc.dram_tensor("woT", [P, 2, C], f32, kind="ExternalInput")
    trid = nc.dram_tensor("trid", [P, P], f32, kind="ExternalInput")
    seld = nc.dram_tensor("seld", [33, P], f32, kind="ExternalInput")
    eyed = nc.dram_tensor("eyed", [P, P], f32, kind="ExternalInput")
    out = nc.dram_tensor("out", [T, C], bf16, kind="ExternalOutput")
    with tile.TileContext(nc) as tc:
        with (
            tc.tile_pool(name="consts", bufs=1) as consts,
            tc.tile_pool(name="resid", bufs=1) as resid,
            tc.tile_pool(name="xload", bufs=4) as xload,
            tc.tile_pool(name="rot", bufs=2) as rot,
            tc.tile_pool(name="small", bufs=4) as small,
            tc.tile_pool(name="exps", bufs=3) as exps,
            tc.tile_pool(name="obp", bufs=2) as obp,
            tc.tile_pool(name="denp", bufs=2) as denp,
            tc.tile_pool(name="psmm", bufs=2, space="PSUM") as psmm,
            tc.tile_pool(name="pssc", bufs=2, space="PSUM") as pssc,
            tc.tile_pool(name="psy", bufs=2, space="PSUM") as psy,
        ):
            # ---- resident loads (wr + xt0 first: they gate the first matmul) ----
            wr_sb = consts.tile([P, KC, NPJ], bf16)
            nc.sync.dma_start(wr_sb[:, 0:4], wr[:, 0:4])
            nc.scalar.dma_start(wr_sb[:, 4:8], wr[:, 4:8])
            xt0 = xload.tile([P, GRP, KC, P], bf16, name="xt0", tag="xt")
            for tl, eng in ((0, nc.gpsimd), (1, nc.scalar), (2, nc.gpsimd), (3, nc.sync)):
                eng.dma_start(xt0[:, tl], xT[:, 0, tl])
            cos_sb = consts.tile([P, NT, H32], bf16)
            nc.sync.dma_start(cos_sb[:], cosd[:])
            sin_sb = consts.tile([P, NT, H32], bf16)
            nc.sync.dma_start(sin_sb[:], sind[:])
            ve3_sb = consts.tile([P, NT, D], bf16)
            tri_sb = consts.tile([P, P], f32)
            wo_sb = consts.tile([P, 2, C], f32r)
            sel_sb = consts.tile([33, P], f32r)
            ident = consts.tile([P, P], f32r)
            zero_sb = consts.tile([P, 1], f32)
            nc.vector.memset(zero_sb[:], 0.0)
            rsq_k = consts.tile([P, 1], u32)
            nc.vector.memset(rsq_k[:], 0x5F3759DF)

            # ---- residents written by the kernel ----
            qT = resid.tile([P, 2, T], f32r)   # [h0|h1] rows, [h2|h3] rows
            kT2 = resid.tile([P, T], f32r)     # kT duplicated in both halves
            v_aug = resid.tile([P, NT, D + 1], f32r)  # v plus ones column
            nc.sync.dma_start(
                v_aug[:, :, D : D + 1],
                trid[0:1, 0:1].unsqueeze(1).to_broadcast((P, NT, 1)).bitcast(f32r),
            )
            yT1 = resid.tile([P, T], f32r)     # normalized y, heads 0|1
            yT2 = resid.tile([P, T], f32r)     # heads 2|3


            def load_x(bi):
                xt = xload.tile([P, GRP, KC, P], bf16, name=f"xt{bi}", tag="xt")
                nc.sync.dma_start(xt[:], xT[:, bi])
                return xt

            xts = {0: xt0}

            def phase1_chunk(bi, tl):
                """Jammed projection matmul for one 128-token chunk."""
                xt = xts[bi]
                pj = psmm.tile([P, 512], f32, tag="mm", name="pj")
                for kc in range(KC):
                    nc.tensor.matmul(
                        pj[:, 0:NPJ],
                        xt[:, tl, kc, :],
                        wr_sb[:, kc, :],
                        start=(kc == 0),
                        stop=(kc == KC - 1),
                    )
                pjg = phase1_chunk.pjgs[bi]
                nc.vector.tensor_copy(pjg[:, tl, :], pj[:, 0:NPJ])

            def phase1_begin(bi):
                phase1_chunk.pjgs[bi] = rot.tile(
                    [P, GRP, NPJ], bf16, tag="pjg", bufs=3, name=f"pjg{bi}"
                )

            def phase1_dve(bi):
                """rope + rms rstd + normalize + gate + ve-gating for group bi."""
                pjg = phase1_chunk.pjgs[bi]
                qkr = rot.tile([P, GRP, 320], bf16, tag="qkr", bufs=1)
                tmp = rot.tile([P, GRP, 160], bf16, tag="tmp", bufs=1)
                sqg = rot.tile([P, GRP, 320], bf16, tag="sqg", bufs=1)
                msg = small.tile([P, GRP * 5], f32, tag="msg")
                rstdg = small.tile([P, GRP * 5], f32, tag="rstdg")
                nwt = small.tile([P, GRP * 5], f32, tag="nwt")
                qkn = rot.tile([P, GRP, 320], f32r, tag="qkn", bufs=2)

                qv5 = pjg[:, :, 0:320].rearrange("p g (h d) -> p g h d", d=D)
                ro5 = qkr[:].rearrange("p g (h d) -> p g h d", d=D)
                t5 = tmp[:].rearrange("p g (h d) -> p g h d", d=H32)
                cs = cos_sb[:, bi * GRP : (bi + 1) * GRP, :]
                sn = sin_sb[:, bi * GRP : (bi + 1) * GRP, :]
                cos5 = cs.unsqueeze(2).broadcast_to([P, GRP, 5, H32])
                sin5 = sn.unsqueeze(2).broadcast_to([P, GRP, 5, H32])
                q1 = qv5[:, :, :, 0:H32]
                q2 = qv5[:, :, :, H32:D]
                nc.vector.tensor_mul(ro5[:, :, :, 0:H32], q1, cos5)
                nc.vector.tensor_mul(t5[:], q2, sin5)
                nc.vector.tensor_add(ro5[:, :, :, 0:H32], ro5[:, :, :, 0:H32], t5[:])
                nc.vector.tensor_mul(ro5[:, :, :, H32:D], q2, cos5)
                nc.vector.tensor_mul(t5[:], q1, sin5)
                nc.vector.tensor_sub(ro5[:, :, :, H32:D], ro5[:, :, :, H32:D], t5[:])

                nc.vector.tensor_mul(sqg[:], qkr[:], qkr[:])
                nc.vector.reduce_sum(
                    msg[:],
                    sqg[:].rearrange("p g (h d) -> p (g h) d", d=D),
                    axis=mybir.AxisListType.X,
                )
                # m = mean + eps; rstd = m^-1/2 by bit-trick seed + two
                # Newton iterations, entirely on DVE (no ACT Ln table).
                nf = GRP * 5
                nc.vector.tensor_scalar(
                    msg[:], msg[:], 1.0 / D, 1e-6, op0=ALU.mult, op1=ALU.add
                )
                rstdu = rstdg[:].bitcast(u32)
                nc.vector.tensor_scalar(
                    rstdu, msg[:].bitcast(u32), 1, None,
                    op0=ALU.logical_shift_right,
                )
                nc.vector.tensor_sub(
                    rstdu, rsq_k[:].broadcast_to([P, nf]).bitcast(u32), rstdu
                )
                for _ in range(1):
                    nc.vector.tensor_mul(nwt[:], msg[:], rstdg[:])
                    nc.vector.tensor_mul(nwt[:], nwt[:], rstdg[:])
                    nc.vector.tensor_scalar(
                        nwt[:], nwt[:], -0.5, 1.5, op0=ALU.mult, op1=ALU.add
                    )
                    nc.vector.tensor_mul(rstdg[:], rstdg[:], nwt[:])
                nc.vector.tensor_mul(
                    qkn[:].rearrange("p g (h d) -> p (g h) d", d=D),
                    qkr[:].rearrange("p g (h d) -> p (g h) d", d=D),
                    rstdg[:].unsqueeze(2).broadcast_to([P, nf, D]),
                )

                # gate r = sigmoid(z) = 0.5 + 0.5*tanh(z/2); ve3 is 3*ve.
                tgg = small.tile([P, GRP], f32, tag="tgg")
                nc.scalar.activation(
                    tgg[:].unsqueeze(2), pjg[:, :, 384:385], AF.Tanh,
                    scale=0.5, bias=zero_sb[:],
                )
                rgg = small.tile([P, GRP], bf16, tag="rgg")
                nc.vector.tensor_scalar(
                    rgg[:], tgg[:], 0.5, 0.5, op0=ALU.mult, op1=ALU.add
                )
                vtg = small.tile([P, GRP, D], bf16, tag="vtg", bufs=1)
                nc.gpsimd.tensor_mul(
                    vtg[:],
                    ve3_sb[:, bi * GRP : (bi + 1) * GRP, :],
                    rgg[:].unsqueeze(2).broadcast_to([P, GRP, D]),
                )
                nc.gpsimd.tensor_add(
                    v_aug[:, bi * GRP : (bi + 1) * GRP, 0:D],
                    pjg[:, :, 320:384],
                    vtg[:],
                )
                phase1b_chunk.qkns[bi] = qkn

            def phase1b_chunk(bi, tl):
                """Transpose one chunk: 2 q-pair transposes + k transpose.
                Group 0's copies ride the (startup-idle) ACT engine so the
                first scores don't queue behind the DVE rope chain."""
                qkn = phase1b_chunk.qkns[bi]
                tc_ = bi * GRP + tl
                tp = psmm.tile([P, 512], f32r, tag="mm", name="tp")
                qknr = qkn[:, tl, :]
                idr = ident[:]
                nc.tensor.transpose(tp[:, 0:P], qknr[:, 0:128], idr)
                nc.tensor.transpose(tp[:, P : 2 * P], qknr[:, 128:256], idr)
                nc.tensor.transpose(tp[0:D, 2 * P : 3 * P], qknr[:, 256:320], idr)
                if bi == 0:
                    nc.scalar.copy(
                        qT[:, :, tc_ * P : (tc_ + 1) * P],
                        tp[:, 0 : 2 * P].rearrange("p (g t) -> p g t", g=2),
                    )
                    nc.scalar.copy(
                        kT2[0:D, tc_ * P : (tc_ + 1) * P], tp[0:D, 2 * P : 3 * P]
                    )
                    if tl == GRP - 1:
                        nc.scalar.copy(
                            kT2[D:P, bi * IB : (bi + 1) * IB],
                            kT2[0:D, bi * IB : (bi + 1) * IB],
                        )
                else:
                    nc.vector.tensor_copy(
                        qT[:, :, tc_ * P : (tc_ + 1) * P],
                        tp[:, 0 : 2 * P].rearrange("p (g t) -> p g t", g=2),
                    )
                    nc.vector.tensor_copy(
                        kT2[0:D, tc_ * P : (tc_ + 1) * P], tp[0:D, 2 * P : 3 * P]
                    )
                    if tl == GRP - 1:
                        nc.vector.tensor_copy(
                            kT2[D:P, bi * IB : (bi + 1) * IB],
                            kT2[0:D, bi * IB : (bi + 1) * IB],
                        )

            def norm_prep(bi, pr):
                """Reciprocal of the pair's denominators + selector matmul
                broadcasting both rows to 128 partitions (needs only den,
                so it can overlap the yT copy that precedes the scale)."""
                dent = phase2.dens[(bi, pr)]
                rct = denp.tile([33, IB], f32, tag="rct", name="rct")
                rr2 = denp.tile([33, IB], f32r, tag="rr2", name="rr2")
                rbp = psmm.tile([P, 512], f32, tag="mm", name="rbp")
                halves = 2 if bi == NBI - 1 else 1
                hw_ = IB // halves
                parts = []
                for hf in range(halves):
                    hs = slice(hf * hw_, (hf + 1) * hw_)
                    bs = slice(bi * IB + hf * hw_, bi * IB + (hf + 1) * hw_)
                    nc.vector.reciprocal_approx_fast(rct[:, hs], dent[:, hs])
                    nc.gpsimd.tensor_copy(rr2[:, hs], rct[:, hs])
                    nc.tensor.matmul(
                        rbp[:, hs], sel_sb[:], rr2[:, hs],
                        start=True, stop=True,
                    )
                    parts.append((hs, bs))
                return rbp, parts

            def norm_scale(bi, pr, prep):
                rbp, parts = prep
                ytp = yT1 if pr == 0 else yT2
                for hs, bs in parts:
                    nc.vector.tensor_mul(ytp[:, bs], ytp[:, bs], rbp[:, hs])

            def norm_pair(bi, pr):
                norm_scale(bi, pr, norm_prep(bi, pr))

            phase1b_chunk.qkns = {}
            phase1_chunk.pjgs = {}

            def phase2(entries, fillers):
                """Scores -> exp(pairs) -> mask -> AV -> normalize for a list
                of (bi, h) entries sharing one staggered pipeline, with
                filler emission interleaved between score pairs."""
                fill_i = 0
                nslots = sum(2 * (bi + 1) for bi, _ in entries)
                stride = max(1, nslots // max(1, len(fillers)))
                slot = 0

                def maybe_fill():
                    nonlocal fill_i, slot
                    slot += 1
                    while fill_i < len(fillers) and slot >= stride * (fill_i + 1):
                        fn = fillers[fill_i]
                        if isinstance(fn, tuple):
                            fn, ready = fn
                            if not ready():
                                break
                        fn()
                        fill_i += 1

                pending = []
                for ei, (bi, h) in enumerate(entries):
                    npair = 2 * (bi + 1)
                    njt = GRP * (bi + 1)
                    rr = D * (h % 2)
                    qTh = qT[rr : rr + D, h // 2, :]
                    kTr = kT2[rr : rr + D, :]
                    yp = psy.tile([D + 1, 512], f32, tag="y", name="yp")

                    def emit_scores(pr):
                        sp = pssc.tile([P, 2, IB], f32, tag="sc", name="sp")
                        ex = exps.tile([P, 2, IB], f32r, tag="ex", name="ex")
                        j0 = 2 * pr
                        dg0 = j0 - GRP * bi
                        split = dg0 >= 0
                        for jj in range(2):
                            jt = j0 + jj
                            dg = jt - GRP * bi
                            lo = max(dg, 0) * P
                            elo = lo
                            nc.tensor.matmul(
                                sp[:, jj, lo:IB],
                                kTr[:, jt * P : (jt + 1) * P],
                                qTh[:, bi * IB + lo : (bi + 1) * IB],
                                start=True,
                                stop=True,
                            )
                            if split:
                                nc.scalar.activation(
                                    ex[:, jj, elo:IB], sp[:, jj, elo:IB], AF.Exp,
                                    scale=SC, bias=zero_sb[:],
                                )
                        if not split:
                            exf = ex[:].rearrange("p a b -> p (a b)")
                            spf = sp[:].rearrange("p a b -> p (a b)")
                            nc.scalar.activation(
                                exf[:], spf[:], AF.Exp, scale=SC, bias=zero_sb[:]
                            )
                        for jj in range(2):
                            dg = j0 + jj - GRP * bi
                            if 0 <= dg < GRP:
                                sl = ex[:, jj, dg * P : (dg + 1) * P]
                                nc.gpsimd.tensor_mul(sl, sl, tri_sb[:])
                        return j0, ex

                    def emit_av(j0, ex, yp=yp, bi=bi, njt=njt):
                        for jj in range(2):
                            jt = j0 + jj
                            dg = jt - GRP * bi
                            lo = max(dg, 0) * P
                            nc.tensor.matmul(
                                yp[:, lo:IB],
                                v_aug[:, jt, :],
                                ex[:, jj, lo:IB],
                                start=(jt == 0),
                                stop=(jt == njt - 1),
                            )

                    blk = slice(bi * IB, (bi + 1) * IB)
                    if h % 2 == 0:
                        dent = denp.tile(
                            [33, IB], f32, tag=f"den{h // 2}", name="dent"
                        )
                        nc.gpsimd.memset(dent[:], 1.0)
                        phase2.dens[(bi, h // 2)] = dent

                    def finalize(h=h, yp=yp, bi=bi, blk=blk):
                        dent = phase2.dens[(bi, h // 2)]
                        drow = 32 * (h % 2)
                        if bi <= 1:
                            # early blocks are PE/DVE-chain bound while ACT
                            # has slack: drain yp on the scalar engine there
                            nc.scalar.copy(
                                dent[drow : drow + 1, :], yp[D : D + 1, :]
                            )
                        else:
                            nc.vector.tensor_copy(
                                dent[drow : drow + 1, :], yp[D : D + 1, :]
                            )
                        prep = norm_prep(bi, h // 2) if h % 2 == 1 else None
                        ytp = yT1 if h < 2 else yT2
                        row = D * (h % 2)
                        if bi <= 1:
                            nc.scalar.copy(ytp[row : row + D, blk], yp[0:D, :])
                        else:
                            nc.vector.tensor_copy(
                                ytp[row : row + D, blk], yp[0:D, :]
                            )
                        if h % 2 == 1:
                            norm_scale(bi, h // 2, prep)
                            phase2.done.add((bi, h // 2))

                    depth = 1 if ei == len(entries) - 1 else 2
                    for pr in range(npair):
                        post = finalize if pr == npair - 1 else None
                        cur = (emit_scores(pr), emit_av, post)
                        while len(pending) > depth:
                            args, av, p_ = pending.pop(0)
                            av(*args)
                            if p_ is not None:
                                p_()
                        pending.append(cur)
                        maybe_fill()
                    if ei == len(entries) - 1:
                        for args, av, p_ in pending:
                            av(*args)
                            if p_ is not None:
                                p_()
                        pending = []
                while fill_i < len(fillers):
                    fn = fillers[fill_i]
                    if isinstance(fn, tuple):
                        fn = fn[0]
                    fn()
                    fill_i += 1

            phase2.dens = {}
            phase2.done = set()

            def norm3_begin(bi):
                norm3_chunk.ob[bi] = obp.tile(
                    [P, GRP, C], bf16, tag="ob", name=f"ob{bi}"
                )

            def norm3_chunk(bi, qsub):
                """Row-sharded Wo for one 128-token chunk + bf16 out staging."""
                ob = norm3_chunk.ob[bi]
                tc_ = bi * GRP + qsub
                chunk = slice(tc_ * P, (tc_ + 1) * P)
                for cb in range(2):
                    po = psmm.tile([P, 512], f32, tag="mm", name="po")
                    nc.tensor.matmul(
                        po[:],
                        yT1[:, chunk],
                        wo_sb[:, 0, cb * 512 : (cb + 1) * 512],
                        start=True,
                        stop=False,
                    )
                    nc.tensor.matmul(
                        po[:],
                        yT2[:, chunk],
                        wo_sb[:, 1, cb * 512 : (cb + 1) * 512],
                        start=False,
                        stop=True,
                    )
                    dst = ob[:, qsub, cb * 512 : (cb + 1) * 512]
                    if cb == 0 or bi >= 2:
                        nc.vector.tensor_copy(dst, po[:])
                    else:
                        nc.scalar.copy(dst, po[:])

            norm3_chunk.ob = {}

            def norm3_dma_chunk(bi, qsub):
                ob = norm3_chunk.ob[bi]
                tc_ = bi * GRP + qsub
                nc.sync.dma_start(
                    out[tc_ * P : (tc_ + 1) * P, :], ob[:, qsub, :]
                )

            def norm3_dma(bi):
                ob = norm3_chunk.ob[bi]
                nc.sync.dma_start(
                    out[bi * IB : (bi + 1) * IB, :].rearrange(
                        "(c p) d -> p c d", p=P
                    ),
                    ob[:],
                )

            # ---- prologue ----
            nc.scalar.dma_start(ve3_sb[:], ve3[:])
            phase1_begin(0)
            for tl in range(GRP):
                phase1_chunk(0, tl)
            phase1_dve(0)
            xts[1] = load_x(1)
            xts[2] = load_x(2)
            xts[3] = load_x(3)
            # non-urgent consts queue behind the x loads
            nc.sync.dma_start(ident[:], eyed[:].bitcast(f32r))
            nc.sync.dma_start(tri_sb[:], trid[:])
            nc.sync.dma_start(wo_sb[:], woT[:].bitcast(f32r))
            nc.sync.dma_start(sel_sb[:], seld[:].bitcast(f32r))
            phase1_begin(1)
            for tl in range(GRP):
                phase1_chunk(1, tl)
            phase1_dve(1)
            phase1_begin(2)
            for tl in range(GRP):
                phase1_chunk(2, tl)
            phase1_begin(3)
            for tl in range(GRP):
                phase1_chunk(3, tl)
            for tl in range(GRP):
                phase1b_chunk(0, tl)

            # ---- pipelined main loop ----
            fillers = []
            for tl in range(GRP):
                fillers.append(lambda t=tl: phase1b_chunk(1, t))
            fillers.append(lambda: phase1_dve(2))
            phase2([(0, h) for h in range(HQ)], fillers)

            fillers = []
            norm3_begin(0)
            for qsub in range(GRP):
                fillers.append(lambda q=qsub: norm3_chunk(0, q))
            fillers.append(lambda: norm3_dma(0))
            for tl in range(GRP):
                fillers.append(lambda t=tl: phase1b_chunk(2, t))
            fillers.append(lambda: phase1_dve(3))
            for tl in range(GRP):
                fillers.append(lambda t=tl: phase1b_chunk(3, t))
            phase2([(1, h) for h in range(HQ)], fillers)

            # groups 2+3 merged at head granularity: averages the ACT-heavy
            # late group against PE-heavy Wo/output work.
            fillers = []
            norm3_begin(1)
            norm3_begin(2)
            for qsub in range(GRP):
                fillers.append(lambda q=qsub: norm3_chunk(1, q))
            fillers.append(lambda: norm3_dma(1))
            for qsub in range(GRP):
                gate = (
                    (lambda: (2, 1) in phase2.done) if qsub < 2
                    else (lambda: (3, 0) in phase2.done)
                )
                fillers.append((lambda q=qsub: norm3_chunk(2, q), gate))
            fillers.append(
                (lambda: norm3_dma(2), lambda: (2, 1) in phase2.done)
            )
            entries = []
            for h in range(HQ):
                entries.append((2, h))
                entries.append((3, h))
            phase2(entries, fillers)

            # ---- tail ----
            norm3_begin(NBI - 1)
            for qsub in range(GRP):
                norm3_chunk(NBI - 1, qsub)
                norm3_dma_chunk(NBI - 1, qsub)
    nc.compile()
    return nc


def make_core_inputs(x, ve, cos, sin, Wq, Wk, Wv, Wo, Wg):
    """Slice full inputs into the 8 per-core input maps (b-major, then group)."""
    b16 = ml_dtypes.bfloat16
    cosf = np.ascontiguousarray(cos[0, :, 0, :], dtype=np.float32)  # [T, 32]
    sinf = np.ascontiguousarray(sin[0, :, 0, :], dtype=np.float32)
    cos_t = np.ascontiguousarray(
        cosf.reshape(NT, P, H32).transpose(1, 0, 2)
    ).astype(b16)
    sin_t = np.ascontiguousarray(
        sinf.reshape(NT, P, H32).transpose(1, 0, 2)
    ).astype(b16)
    tri = (np.arange(P)[:, None] <= np.arange(P)[None, :]).astype(np.float32)
    sel = np.zeros((33, P), np.float32)
    sel[0, 0:D] = 1.0
    sel[32, D:P] = 1.0
    in_maps = []
    for c in range(8):
        b, g = c // N_KV_HEAD, c % N_KV_HEAD
        xb = np.ascontiguousarray(x[b].T, dtype=np.float32)  # [C, T]
        x_t = np.ascontiguousarray(
            xb.reshape(KC, P, NBI, GRP, P).transpose(1, 2, 3, 0, 4)
        ).astype(b16)  # [P, NBI, GRP, KC, P]
        wq = Wq[g * 256 : (g + 1) * 256, :]           # [256, C]
        wk = Wk[g * D : (g + 1) * D, :]               # [64, C]
        wv = Wv[g * D : (g + 1) * D, :]
        wg_col = np.zeros((C, 1), np.float32)
        wg_col[:12, 0] = Wg[g]
        wrc = np.concatenate(
            [wq.T, wk.T, wv.T, wg_col, np.zeros((C, 1), np.float32)], axis=1
        ).astype(np.float32)                          # [C, 386]
        wr_t = np.ascontiguousarray(
            wrc.reshape(KC, P, NPJ).transpose(1, 0, 2)
        ).astype(b16)                                 # [P, KC, 386]
        ve3 = (3.0 * ve[b, :, g * D : (g + 1) * D]).astype(np.float32)
        ve3_t = np.ascontiguousarray(
            ve3.reshape(NT, P, D).transpose(1, 0, 2)
        ).astype(b16)                                 # [P, NT, 64]
        woc = np.ascontiguousarray(
            Wo[:, g * 256 : (g + 1) * 256].T, dtype=np.float32
        )                                             # [256, C]
        wo_t = np.ascontiguousarray(
            woc.reshape(2, P, C).transpose(1, 0, 2)
        )                                             # [P, 2, C] f32
        in_maps.append(
            {
                "xT": x_t,
                "wr": wr_t,
                "cosd": cos_t,
                "sind": sin_t,
                "ve3": ve3_t,
                "woT": wo_t,
                "trid": tri,
                "seld": sel,
                "eyed": np.eye(P, dtype=np.float32),
            }
        )
    return in_maps


_PROGRAM = None


def kernel(x, ve, cos, sin, Wq, Wk, Wv, Wo, Wg, _trace=False):
    from concourse.bass_utils import run_bass_kernel_spmd

    x, ve, cos, sin, Wq, Wk, Wv, Wo, Wg = (
        np.asarray(a, dtype=np.float32)
        for a in (x, ve, cos, sin, Wq, Wk, Wv, Wo, Wg)
    )
    global _PROGRAM
    if _PROGRAM is None:
        _PROGRAM = build_program()
    nc = _PROGRAM
    in_maps = make_core_inputs(x, ve, cos, sin, Wq, Wk, Wv, Wo, Wg)
    res = run_bass_kernel_spmd(nc, in_maps, list(range(8)), trace=_trace)
    outs = [r["out"] for r in res.results]
    full = np.zeros((B, T, C), np.float32)
    for c in range(8):
        full[c // N_KV_HEAD] += np.asarray(outs[c], dtype=np.float32)
    if _trace:
        kernel.last_results = res
    return full


# revision 9
# speedup vs baseline: 1.0431x; 1.0431x over previous
"""Trainium2 Bass kernel for nn_Attention_5299989643989.

GQA attention forward (B=2, T=2048, C=1024, 16 q heads / 4 kv heads, D=64)
with value-embedding gating, rotary embedding, qk rms-norm, causal softmax.

Sharding: 8 cores = batch (2) x kv-head-group (4).  Each core computes its
4 q heads / 1 kv head end-to-end plus the Wo row-shard partial output; the
host sums the 4 partials per batch (the Wo all-reduce, done at unshard).

Structure (all matmuls f32r: 16-bit weights would emit one Ldweights per
matmul, ~71ns of PE sequencer each, which measured slower than the f32r
penalty; PSUM accumulation groups must be emitted contiguously -- the PE
corrupts interleaved open groups in one bank, though nesting a whole group
inside another group in a DIFFERENT bank is fine):
  inputs:  host pre-tiles everything so each DMA is 128 contiguous
           descriptors; x and the jammed q|k|v|gate weight block are bf16
           (halves the DMA-bound startup), cos/sin/ve bf16, out staged bf16.
  phase1:  per 128-token chunk one jammed projection matmul accumulated
           over C in PSUM -> bf16 SBUF; per group: rope on packed bf16 DVE
           ops (2x mode), sum-of-squares + bit-trick Newton rsqrt for the
           rms-norm, sigmoid gate via Tanh (keeps the Exp ACT table
           resident), ve-gating on GPSIMD; f32r transposes (q pairs + k
           with row duplication so odd heads run at partition base 64).
  phase2:  per (block, head): score matmuls emitted in PAIRS into a
           two-bank PSUM tile, ONE exp over [128, 1024] for off-diagonal
           pairs (split per tile on the diagonal to skip masked columns),
           triangular mask on GPSIMD, AV accumulation yp[65, 512] with a
           ones column producing softmax denominators for free.  The AV of
           pair p trails the scores of pair p+2 (software stagger), and the
           pipeline runs CONTINUOUSLY across heads and across the merged
           blocks 2+3 (head-interleaved to average the ACT-heavy late
           blocks against PE-heavy Wo/output work).
  norm:    per head pair one [33, 512] reciprocal_approx_fast (rows 0/32;
           base-0 only -- the op miscomputes at shifted partition bases),
           one selector matmul broadcasts both rows to 128 partitions,
           one in-place scale of the feature-major y.
  out:     row-sharded Wo per 128-token chunk, PSUM -> bf16 staging split
           across DVE/ACT, one big output DMA per 512-token block.
  The whole thing is emission-software-pipelined: projection chunks for
  ALL groups run in the prologue while x blocks stream in; transposes,
  Wo/output work and the next groups' DVE chains are interleaved between
  phase-2 score pairs as "fillers" with readiness gates.
"""

import numpy as np
import ml_dtypes

import concourse.bacc as bacc
import concourse.bass as bass
import concourse.tile as tile
from concourse import mybir

f32 = mybir.dt.float32
f32r = mybir.dt.float32r
bf16 = mybir.dt.bfloat16
u32 = mybir.dt.uint32
AF = mybir.ActivationFunctionType
ALU = mybir.AluOpType

B, T, C = 2, 2048, 1024
N_HEAD, N_KV_HEAD, D = 16, 4, 64
HQ = N_HEAD // N_KV_HEAD  # q heads per core = 4
P = 128
NT = T // P       # 16 token chunks
KC = C // P       # 8 contraction chunks
IB = 512          # query block
NBI = T // IB     # 4 query blocks
GRP = IB // P     # 4 token chunks per query block
SC = 1.2 * 1.2 / 8.0  # folded qk scale: rms 1.2 factors * 1/sqrt(64)
H32 = D // 2
NPJ = 386         # q(256) | k(64) | v(64) | gate(1) | pad(1)


def build_program():
    nc = bacc.Bacc("TRN2", target_bir_lowering=False, debug=False, num_devices=8)

    xT = nc.dram_tensor("xT", [P, NBI, GRP, KC, P], bf16, kind="ExternalInput")
    wr = nc.dram_tensor("wr", [P, KC, NPJ], bf16, kind="ExternalInput")
    cosd = nc.dram_tensor("cosd", [P, NT, H32], bf16, kind="ExternalInput")
    sind = nc.dram_tensor("sind", [P, NT, H32], bf16, kind="ExternalInput")
    ve3 = nc.dram_tensor("ve3", [P, NT, D], bf16, kind="ExternalInput")
    woT = nc.dram_tensor("woT", [P, 2, C], f32, kind="ExternalInput")
    trid = nc.dram_tensor("trid", [P, P], f32, kind="ExternalInput")
    seld = nc.dram_tensor("seld", [33, P], f32, kind="ExternalInput")
    eyed = nc.dram_tensor("eyed", [P, P], f32, kind="ExternalInput")
    out = nc.dram_tensor("out", [T, C], bf16, kind="ExternalOutput")
    with tile.TileContext(nc) as tc:
        with (
            tc.tile_pool(name="consts", bufs=1) as consts,
            tc.tile_pool(name="resid", bufs=1) as resid,
            tc.tile_pool(name="xload", bufs=4) as xload,
            tc.tile_pool(name="rot", bufs=2) as rot,
            tc.tile_pool(name="small", bufs=4) as small,
            tc.tile_pool(name="exps", bufs=3) as exps,
            tc.tile_pool(name="obp", bufs=2) as obp,
            tc.tile_pool(name="denp", bufs=2) as denp,
            tc.tile_pool(name="psmm", bufs=2, space="PSUM") as psmm,
            tc.tile_pool(name="pssc", bufs=2, space="PSUM") as pssc,
            tc.tile_pool(name="psy", bufs=2, space="PSUM") as psy,
        ):
            # ---- resident loads (wr + xt0 first: they gate the first matmul) ----
            wr_sb = consts.tile([P, KC, NPJ], bf16)
            nc.sync.dma_start(wr_sb[:, 0:4], wr[:, 0:4])
            nc.scalar.dma_start(wr_sb[:, 4:8], wr[:, 4:8])
            xt0 = xload.tile([P, GRP, KC, P], bf16, name="xt0", tag="xt")
            for tl, eng in ((0, nc.gpsimd), (1, nc.scalar), (2, nc.gpsimd), (3, nc.sync)):
                eng.dma_start(xt0[:, tl], xT[:, 0, tl])
            cos_sb = consts.tile([P, NT, H32], bf16)
            nc.sync.dma_start(cos_sb[:], cosd[:])
            sin_sb = consts.tile([P, NT, H32], bf16)
            nc.sync.dma_start(sin_sb[:], sind[:])
            ve3_sb = consts.tile([P, NT, D], bf16)
            tri_sb = consts.tile([P, P], f32)
            wo_sb = consts.tile([P, 2, C], f32r)
            sel_sb = consts.tile([33, P], f32r)
            ident = consts.tile([P, P], f32r)
            zero_sb = consts.tile([P, 1], f32)
            nc.vector.memset(zero_sb[:], 0.0)
            rsq_k = consts.tile([P, 1], u32)
            nc.vector.memset(rsq_k[:], 0x5F3759DF)

            # ---- residents written by the kernel ----
            qT = resid.tile([P, 2, T], f32r)   # [h0|h1] rows, [h2|h3] rows
            kT2 = resid.tile([P, T], f32r)     # kT duplicated in both halves
            v_aug = resid.tile([P, NT, D + 1], f32r)  # v plus ones column
            nc.sync.dma_start(
                v_aug[:, :, D : D + 1],
                trid[0:1, 0:1].unsqueeze(1).to_broadcast((P, NT, 1)).bitcast(f32r),
            )
            yT1 = resid.tile([P, T], f32r)     # normalized y, heads 0|1
            yT2 = resid.tile([P, T], f32r)     # heads 2|3


            def load_x(bi):
                xt = xload.tile([P, GRP, KC, P], bf16, name=f"xt{bi}", tag="xt")
                nc.sync.dma_start(xt[:], xT[:, bi])
                return xt

            xts = {0: xt0}

            def phase1_chunk(bi, tl):
                """Jammed projection matmul for one 128-token chunk."""
                xt = xts[bi]
                pj = psmm.tile([P, 512], f32, tag="mm", name="pj")
                for kc in range(KC):
                    nc.tensor.matmul(
                        pj[:, 0:NPJ],
                        xt[:, tl, kc, :],
                        wr_sb[:, kc, :],
                        start=(kc == 0),
                        stop=(kc == KC - 1),
                    )
                pjg = phase1_chunk.pjgs[bi]
                nc.vector.tensor_copy(pjg[:, tl, :], pj[:, 0:NPJ])

            def phase1_begin(bi):
                phase1_chunk.pjgs[bi] = rot.tile(
                    [P, GRP, NPJ], bf16, tag="pjg", bufs=3, name=f"pjg{bi}"
                )

            def phase1_dve(bi):
                """rope + rms rstd + normalize + gate + ve-gating for group bi."""
                pjg = phase1_chunk.pjgs[bi]
                qkr = rot.tile([P, GRP, 320], bf16, tag="qkr", bufs=1)
                tmp = rot.tile([P, GRP, 160], bf16, tag="tmp", bufs=1)
                sqg = rot.tile([P, GRP, 320], bf16, tag="sqg", bufs=1)
                msg = small.tile([P, GRP * 5], f32, tag="msg")
                rstdg = small.tile([P, GRP * 5], f32, tag="rstdg")
                nwt = small.tile([P, GRP * 5], f32, tag="nwt")
                qkn = rot.tile([P, GRP, 320], f32r, tag="qkn", bufs=2)

                qv5 = pjg[:, :, 0:320].rearrange("p g (h d) -> p g h d", d=D)
                ro5 = qkr[:].rearrange("p g (h d) -> p g h d", d=D)
                t5 = tmp[:].rearrange("p g (h d) -> p g h d", d=H32)
                cs = cos_sb[:, bi * GRP : (bi + 1) * GRP, :]
                sn = sin_sb[:, bi * GRP : (bi + 1) * GRP, :]
                cos5 = cs.unsqueeze(2).broadcast_to([P, GRP, 5, H32])
                sin5 = sn.unsqueeze(2).broadcast_to([P, GRP, 5, H32])
                q1 = qv5[:, :, :, 0:H32]
                q2 = qv5[:, :, :, H32:D]
                nc.vector.tensor_mul(ro5[:, :, :, 0:H32], q1, cos5)
                nc.vector.tensor_mul(t5[:], q2, sin5)
                nc.vector.tensor_add(ro5[:, :, :, 0:H32], ro5[:, :, :, 0:H32], t5[:])
                nc.vector.tensor_mul(ro5[:, :, :, H32:D], q2, cos5)
                nc.vector.tensor_mul(t5[:], q1, sin5)
                nc.vector.tensor_sub(ro5[:, :, :, H32:D], ro5[:, :, :, H32:D], t5[:])

                nc.vector.tensor_mul(sqg[:], qkr[:], qkr[:])
                nc.vector.reduce_sum(
                    msg[:],
                    sqg[:].rearrange("p g (h d) -> p (g h) d", d=D),
                    axis=mybir.AxisListType.X,
                )
                # m = mean + eps; rstd = m^-1/2 by bit-trick seed + two
                # Newton iterations, entirely on DVE (no ACT Ln table).
                nf = GRP * 5
                nc.vector.tensor_scalar(
                    msg[:], msg[:], 1.0 / D, 1e-6, op0=ALU.mult, op1=ALU.add
                )
                rstdu = rstdg[:].bitcast(u32)
                nc.vector.tensor_scalar(
                    rstdu, msg[:].bitcast(u32), 1, None,
                    op0=ALU.logical_shift_right,
                )
                nc.vector.tensor_sub(
                    rstdu, rsq_k[:].broadcast_to([P, nf]).bitcast(u32), rstdu
                )
                for _ in range(1):
                    nc.vector.tensor_mul(nwt[:], msg[:], rstdg[:])
                    nc.vector.tensor_mul(nwt[:], nwt[:], rstdg[:])
                    nc.vector.tensor_scalar(
                        nwt[:], nwt[:], -0.5, 1.5, op0=ALU.mult, op1=ALU.add
                    )
                    nc.vector.tensor_mul(rstdg[:], rstdg[:], nwt[:])
                nc.vector.tensor_mul(
                    qkn[:].rearrange("p g (h d) -> p (g h) d", d=D),
                    qkr[:].rearrange("p g (h d) -> p (g h) d", d=D),
                    rstdg[:].unsqueeze(2).broadcast_to([P, nf, D]),
                )

                # gate r = sigmoid(z) = 0.5 + 0.5*tanh(z/2); ve3 is 3*ve.
                tgg = small.tile([P, GRP], f32, tag="tgg")
                nc.scalar.activation(
                    tgg[:].unsqueeze(2), pjg[:, :, 384:385], AF.Tanh,
                    scale=0.5, bias=zero_sb[:],
                )
                rgg = small.tile([P, GRP], bf16, tag="rgg")
                nc.vector.tensor_scalar(
                    rgg[:], tgg[:], 0.5, 0.5, op0=ALU.mult, op1=ALU.add
                )
                vtg = small.tile([P, GRP, D], bf16, tag="vtg", bufs=1)
                nc.gpsimd.tensor_mul(
                    vtg[:],
                    ve3_sb[:, bi * GRP : (bi + 1) * GRP, :],
                    rgg[:].unsqueeze(2).broadcast_to([P, GRP, D]),
                )
                nc.gpsimd.tensor_add(
                    v_aug[:, bi * GRP : (bi + 1) * GRP, 0:D],
                    pjg[:, :, 320:384],
                    vtg[:],
                )
                phase1b_chunk.qkns[bi] = qkn

            def phase1b_chunk(bi, tl):
                """Transpose one chunk: 2 q-pair transposes + k transpose.
                Group 0's copies ride the (startup-idle) ACT engine so the
                first scores don't queue behind the DVE rope chain."""
                qkn = phase1b_chunk.qkns[bi]
                tc_ = bi * GRP + tl
                tp = psmm.tile([P, 512], f32r, tag="mm", name="tp")
                qknr = qkn[:, tl, :]
                idr = ident[:]
                nc.tensor.transpose(tp[:, 0:P], qknr[:, 0:128], idr)
                nc.tensor.transpose(tp[:, P : 2 * P], qknr[:, 128:256], idr)
                nc.tensor.transpose(tp[0:D, 2 * P : 3 * P], qknr[:, 256:320], idr)
                if bi == 0:
                    nc.scalar.copy(
                        qT[:, :, tc_ * P : (tc_ + 1) * P],
                        tp[:, 0 : 2 * P].rearrange("p (g t) -> p g t", g=2),
                    )
                    nc.scalar.copy(
                        kT2[0:D, tc_ * P : (tc_ + 1) * P], tp[0:D, 2 * P : 3 * P]
                    )
                    if tl == GRP - 1:
                        nc.scalar.copy(
                            kT2[D:P, bi * IB : (bi + 1) * IB],
                            kT2[0:D, bi * IB : (bi + 1) * IB],
                        )
                else:
                    nc.vector.tensor_copy(
                        qT[:, :, tc_ * P : (tc_ + 1) * P],
                        tp[:, 0 : 2 * P].rearrange("p (g t) -> p g t", g=2),
                    )
                    nc.vector.tensor_copy(
                        kT2[0:D, tc_ * P : (tc_ + 1) * P], tp[0:D, 2 * P : 3 * P]
                    )
                    if tl == GRP - 1:
                        nc.vector.tensor_copy(
                            kT2[D:P, bi * IB : (bi + 1) * IB],
                            kT2[0:D, bi * IB : (bi + 1) * IB],
                        )

            def norm_prep(bi, pr):
                """Reciprocal of the pair's denominators + selector matmul
                broadcasting both rows to 128 partitions (needs only den,
                so it can overlap the yT copy that precedes the scale)."""
                dent = phase2.dens[(bi, pr)]
                rct = denp.tile([33, IB], f32, tag="rct", name="rct")
                rr2 = denp.tile([33, IB], f32r, tag="rr2", name="rr2")
                rbp = psmm.tile([P, 512], f32, tag="mm", name="rbp")
                halves = 2 if bi == NBI - 1 else 1
                hw_ = IB // halves
                parts = []
                for hf in range(halves):
                    hs = slice(hf * hw_, (hf + 1) * hw_)
                    bs = slice(bi * IB + hf * hw_, bi * IB + (hf + 1) * hw_)
                    nc.vector.reciprocal_approx_fast(rct[:, hs], dent[:, hs])
                    nc.gpsimd.tensor_copy(rr2[:, hs], rct[:, hs])
                    nc.tensor.matmul(
                        rbp[:, hs], sel_sb[:], rr2[:, hs],
                        start=True, stop=True,
                    )
                    parts.append((hs, bs))
                return rbp, parts

            def norm_scale(bi, pr, prep):
                rbp, parts = prep
                ytp = yT1 if pr == 0 else yT2
                for hs, bs in parts:
                    nc.vector.tensor_mul(ytp[:, bs], ytp[:, bs], rbp[:, hs])

            def norm_pair(bi, pr):
                norm_scale(bi, pr, norm_prep(bi, pr))

            phase1b_chunk.qkns = {}
            phase1_chunk.pjgs = {}

            def phase2(entries, fillers):
                """Scores -> exp(pairs) -> mask -> AV -> normalize for a list
                of (bi, h) entries sharing one staggered pipeline, with
                filler emission interleaved between score pairs."""
                fill_i = 0
                nslots = sum(2 * (bi + 1) for bi, _ in entries)
                stride = max(1, nslots // max(1, len(fillers)))
                slot = 0

                def maybe_fill():
                    nonlocal fill_i, slot
                    slot += 1
                    while fill_i < len(fillers) and slot >= stride * (fill_i + 1):
                        fn = fillers[fill_i]
                        if isinstance(fn, tuple):
                            fn, ready = fn
                            if not ready():
                                break
                        fn()
                        fill_i += 1

                pending = []
                for ei, (bi, h) in enumerate(entries):
                    npair = 2 * (bi + 1)
                    njt = GRP * (bi + 1)
                    rr = D * (h % 2)
                    qTh = qT[rr : rr + D, h // 2, :]
                    kTr = kT2[rr : rr + D, :]
                    yp = psy.tile([D + 1, 512], f32, tag="y", name="yp")

                    def emit_scores(pr):
                        sp = pssc.tile([P, 2, IB], f32, tag="sc", name="sp")
                        ex = exps.tile([P, 2, IB], f32r, tag="ex", name="ex")
                        j0 = 2 * pr
                        dg0 = j0 - GRP * bi
                        split = dg0 >= 0
                        for jj in range(2):
                            jt = j0 + jj
                            dg = jt - GRP * bi
                            lo = max(dg, 0) * P
                            elo = lo
                            nc.tensor.matmul(
                                sp[:, jj, lo:IB],
                                kTr[:, jt * P : (jt + 1) * P],
                                qTh[:, bi * IB + lo : (bi + 1) * IB],
                                start=True,
                                stop=True,
                            )
                            if split:
                                nc.scalar.activation(
                                    ex[:, jj, elo:IB], sp[:, jj, elo:IB], AF.Exp,
                                    scale=SC, bias=zero_sb[:],
                                )
                        if not split:
                            exf = ex[:].rearrange("p a b -> p (a b)")
                            spf = sp[:].rearrange("p a b -> p (a b)")
                            nc.scalar.activation(
                                exf[:], spf[:], AF.Exp, scale=SC, bias=zero_sb[:]
                            )
                        for jj in range(2):
                            dg = j0 + jj - GRP * bi
                            if 0 <= dg < GRP:
                                sl = ex[:, jj, dg * P : (dg + 1) * P]
                                nc.gpsimd.tensor_mul(sl, sl, tri_sb[:])
                        return j0, ex

                    def emit_av(j0, ex, yp=yp, bi=bi, njt=njt):
                        for jj in range(2):
                            jt = j0 + jj
                            dg = jt - GRP * bi
                            lo = max(dg, 0) * P
                            nc.tensor.matmul(
                                yp[:, lo:IB],
                                v_aug[:, jt, :],
                                ex[:, jj, lo:IB],
                                start=(jt == 0),
                                stop=(jt == njt - 1),
                            )

                    blk = slice(bi * IB, (bi + 1) * IB)
                    if h % 2 == 0:
                        dent = denp.tile(
                            [33, IB], f32, tag=f"den{h // 2}", name="dent"
                        )
                        nc.gpsimd.memset(dent[:], 1.0)
                        phase2.dens[(bi, h // 2)] = dent

                    def finalize(h=h, yp=yp, bi=bi, blk=blk):
                        dent = phase2.dens[(bi, h // 2)]
                        drow = 32 * (h % 2)
                        nc.vector.tensor_copy(
                            dent[drow : drow + 1, :], yp[D : D + 1, :]
                        )
                        prep = norm_prep(bi, h // 2) if h % 2 == 1 else None
                        ytp = yT1 if h < 2 else yT2
                        row = D * (h % 2)
                        nc.vector.tensor_copy(ytp[row : row + D, blk], yp[0:D, :])
                        if h % 2 == 1:
                            norm_scale(bi, h // 2, prep)
                            phase2.done.add((bi, h // 2))

                    depth = 1 if ei == len(entries) - 1 else 2
                    for pr in range(npair):
                        post = finalize if pr == npair - 1 else None
                        cur = (emit_scores(pr), emit_av, post)
                        while len(pending) > depth:
                            args, av, p_ = pending.pop(0)
                            av(*args)
                            if p_ is not None:
                                p_()
                        pending.append(cur)
                        maybe_fill()
                    if ei == len(entries) - 1:
                        for args, av, p_ in pending:
                            av(*args)
                            if p_ is not None:
                                p_()
                        pending = []
                while fill_i < len(fillers):
                    fn = fillers[fill_i]
                    if isinstance(fn, tuple):
                        fn = fn[0]
                    fn()
                    fill_i += 1

            phase2.dens = {}
            phase2.done = set()

            def norm3_begin(bi):
                norm3_chunk.ob[bi] = obp.tile(
                    [P, GRP, C], bf16, tag="ob", name=f"ob{bi}"
                )

            def norm3_chunk(bi, qsub):
                """Row-sharded Wo for one 128-token chunk + bf16 out staging."""
                ob = norm3_chunk.ob[bi]
                tc_ = bi * GRP + qsub
                chunk = slice(tc_ * P, (tc_ + 1) * P)
                for cb in range(2):
                    po = psmm.tile([P, 512], f32, tag="mm", name="po")
                    nc.tensor.matmul(
                        po[:],
                        yT1[:, chunk],
                        wo_sb[:, 0, cb * 512 : (cb + 1) * 512],
                        start=True,
                        stop=False,
                    )
                    nc.tensor.matmul(
                        po[:],
                        yT2[:, chunk],
                        wo_sb[:, 1, cb * 512 : (cb + 1) * 512],
                        start=False,
                        stop=True,
                    )
                    dst = ob[:, qsub, cb * 512 : (cb + 1) * 512]
                    if cb == 0 or bi >= 2:
                        nc.vector.tensor_copy(dst, po[:])
                    else:
                        nc.scalar.copy(dst, po[:])

            norm3_chunk.ob = {}

            def norm3_dma_chunk(bi, qsub):
                ob = norm3_chunk.ob[bi]
                tc_ = bi * GRP + qsub
                nc.sync.dma_start(
                    out[tc_ * P : (tc_ + 1) * P, :], ob[:, qsub, :]
                )

            def norm3_dma(bi):
                ob = norm3_chunk.ob[bi]
                nc.sync.dma_start(
                    out[bi * IB : (bi + 1) * IB, :].rearrange(
                        "(c p) d -> p c d", p=P
                    ),
                    ob[:],
                )

            # ---- prologue ----
            nc.scalar.dma_start(ve3_sb[:], ve3[:])
            phase1_begin(0)
            for tl in range(GRP):
                phase1_chunk(0, tl)
            phase1_dve(0)
            xts[1] = load_x(1)
            xts[2] = load_x(2)
            xts[3] = load_x(3)
            # non-urgent consts queue behind the x loads
            nc.sync.dma_start(ident[:], eyed[:].bitcast(f32r))
            nc.sync.dma_start(tri_sb[:], trid[:])
            nc.sync.dma_start(wo_sb[:], woT[:].bitcast(f32r))
            nc.sync.dma_start(sel_sb[:], seld[:].bitcast(f32r))
            phase1_begin(1)
            for tl in range(GRP):
                phase1_chunk(1, tl)
            phase1_dve(1)
            phase1_begin(2)
            for tl in range(GRP):
                phase1_chunk(2, tl)
            phase1_begin(3)
            for tl in range(GRP):
                phase1_chunk(3, tl)
            for tl in range(GRP):
                phase1b_chunk(0, tl)

            # ---- pipelined main loop ----
            fillers = []
            for tl in range(GRP):
                fillers.append(lambda t=tl: phase1b_chunk(1, t))
            fillers.append(lambda: phase1_dve(2))
            phase2([(0, h) for h in range(HQ)], fillers)

            fillers = []
            norm3_begin(0)
            for qsub in range(GRP):
                fillers.append(lambda q=qsub: norm3_chunk(0, q))
            fillers.append(lambda: norm3_dma(0))
            for tl in range(GRP):
                fillers.append(lambda t=tl: phase1b_chunk(2, t))
            fillers.append(lambda: phase1_dve(3))
            for tl in range(GRP):
                fillers.append(lambda t=tl: phase1b_chunk(3, t))
            phase2([(1, h) for h in range(HQ)], fillers)

            # groups 2+3 merged at head granularity: averages the ACT-heavy
            # late group against PE-heavy Wo/output work.
            fillers = []
            norm3_begin(1)
            norm3_begin(2)
            for qsub in range(GRP):
                fillers.append(lambda q=qsub: norm3_chunk(1, q))
            fillers.append(lambda: norm3_dma(1))
            for qsub in range(GRP):
                gate = (
                    (lambda: (2, 1) in phase2.done) if qsub < 2
                    else (lambda: (3, 0) in phase2.done)
                )
                fillers.append((lambda q=qsub: norm3_chunk(2, q), gate))
            fillers.append(
                (lambda: norm3_dma(2), lambda: (2, 1) in phase2.done)
            )
            entries = []
            for h in range(HQ):
                entries.append((2, h))
                entries.append((3, h))
            phase2(entries, fillers)

            # ---- tail ----
            norm3_begin(NBI - 1)
            for qsub in range(GRP):
                norm3_chunk(NBI - 1, qsub)
                norm3_dma_chunk(NBI - 1, qsub)
    nc.compile()
    return nc


def make_core_inputs(x, ve, cos, sin, Wq, Wk, Wv, Wo, Wg):
    """Slice full inputs into the 8 per-core input maps (b-major, then group)."""
    b16 = ml_dtypes.bfloat16
    cosf = np.ascontiguousarray(cos[0, :, 0, :], dtype=np.float32)  # [T, 32]
    sinf = np.ascontiguousarray(sin[0, :, 0, :], dtype=np.float32)
    cos_t = np.ascontiguousarray(
        cosf.reshape(NT, P, H32).transpose(1, 0, 2)
    ).astype(b16)
    sin_t = np.ascontiguousarray(
        sinf.reshape(NT, P, H32).transpose(1, 0, 2)
    ).astype(b16)
    tri = (np.arange(P)[:, None] <= np.arange(P)[None, :]).astype(np.float32)
    sel = np.zeros((33, P), np.float32)
    sel[0, 0:D] = 1.0
    sel[32, D:P] = 1.0
    in_maps = []
    for c in range(8):
        b, g = c // N_KV_HEAD, c % N_KV_HEAD
        xb = np.ascontiguousarray(x[b].T, dtype=np.float32)  # [C, T]
        x_t = np.ascontiguousarray(
            xb.reshape(KC, P, NBI, GRP, P).transpose(1, 2, 3, 0, 4)
        ).astype(b16)  # [P, NBI, GRP, KC, P]
        wq = Wq[g * 256 : (g + 1) * 256, :]           # [256, C]
        wk = Wk[g * D : (g + 1) * D, :]               # [64, C]
        wv = Wv[g * D : (g + 1) * D, :]
        wg_col = np.zeros((C, 1), np.float32)
        wg_col[:12, 0] = Wg[g]
        wrc = np.concatenate(
            [wq.T, wk.T, wv.T, wg_col, np.zeros((C, 1), np.float32)], axis=1
        ).astype(np.float32)                          # [C, 386]
        wr_t = np.ascontiguousarray(
            wrc.reshape(KC, P, NPJ).transpose(1, 0, 2)
        ).astype(b16)                                 # [P, KC, 386]
        ve3 = (3.0 * ve[b, :, g * D : (g + 1) * D]).astype(np.float32)
        ve3_t = np.ascontiguousarray(
            ve3.reshape(NT, P, D).transpose(1, 0, 2)
        ).astype(b16)                                 # [P, NT, 64]
        woc = np.ascontiguousarray(
            Wo[:, g * 256 : (g + 1) * 256].T, dtype=np.float32
        )                                             # [256, C]
        wo_t = np.ascontiguousarray(
            woc.reshape(2, P, C).transpose(1, 0, 2)
        )                                             # [P, 2, C] f32
        in_maps.append(
            {
                "xT": x_t,
                "wr": wr_t,
                "cosd": cos_t,
                "sind": sin_t,
                "ve3": ve3_t,
                "woT": wo_t,
                "trid": tri,
                "seld": sel,
                "eyed": np.eye(P, dtype=np.float32),
            }
        )
    return in_maps


_PROGRAM = None


def kernel(x, ve, cos, sin, Wq, Wk, Wv, Wo, Wg, _trace=False):
    from concourse.bass_utils import run_bass_kernel_spmd

    x, ve, cos, sin, Wq, Wk, Wv, Wo, Wg = (
        np.asarray(a, dtype=np.float32)
        for a in (x, ve, cos, sin, Wq, Wk, Wv, Wo, Wg)
    )
    global _PROGRAM
    if _PROGRAM is None:
        _PROGRAM = build_program()
    nc = _PROGRAM
    in_maps = make_core_inputs(x, ve, cos, sin, Wq, Wk, Wv, Wo, Wg)
    res = run_bass_kernel_spmd(nc, in_maps, list(range(8)), trace=_trace)
    outs = [r["out"] for r in res.results]
    full = np.zeros((B, T, C), np.float32)
    for c in range(8):
        full[c // N_KV_HEAD] += np.asarray(outs[c], dtype=np.float32)
    if _trace:
        kernel.last_results = res
    return full
